# revision 23
# baseline (speedup 1.0000x reference)
"""Trainium2 Bass kernel for nn_BERTEmbedding_65274912964883.

out[b, l, :] = token_table[seq[b, l]]
             + mean_{g in genres(seq[b, l])} genre_table[g]
             + pos_table[l]

Strategy (8 NeuronCores, SPMD, no collectives):
  - Data-parallel over batch: 256 sequences -> 32 per core (6400 tokens/core).
  - The genre mean is a function of token id only, so it is folded into the
    token table at weight-prep time: ctab[v, :] = token_table[v] +
    mean_genre[v], bf16, 256B rows.
  - The 6400-row random gather is descriptor-execution bound (~3.4ns/desc
    with all 4 SWDGE queues; ~11ns on one). The generic indirect-DMA path
    is stuck on queue 0 by ucode, so the gather uses the vectorized
    dma_gather ucode instead, spread across queues 0-3.
  - dma_gather takes int16 indices, so the vocab is split into 8 windows of
    12800 rows; the host buckets each core's token ids by window (stable
    order) and ships per-window int16 index lists, each padded to exactly
    NVALID=960 entries with spread-out dummy indices. num_idxs_reg must
    EQUAL the list's valid count or the ucode wedges; runtime counts via
    value_load work (bare only - min_val/max_val bounds wedge) but cost
    ~1us of Pool-sequencer time each, so fixed fully-padded lists win.
    Same-row dummy fetches would serialize on one HBM bank, hence the
    spread. One gather instruction per window, <= 1024 indices each
    (>1024 rows/instruction wedges the SWDGE ring).
  - The positional term is added on-device by DVE from a host-staged
    per-bucket-slot pos tensor; its load and the output stores ride the
    HWDGE queues (sync/scalar engines), overlapping the SWDGE gathers.
  - Device output is in bucket order; the host un-permutes (pure data
    movement, like the batch unshard) and converts bf16 -> f32.
"""

import numpy as np
import ml_dtypes
from contextlib import ExitStack

import concourse.bacc as bacc
import concourse.bass as bass
import concourse.mybir as mybir
from concourse.bass_utils import run_bass_kernel_spmd
from concourse.library_config import mlp

VOCAB = 100000
D = 128
MAXG = 8
B, L = 256, 200
NCORES = 8
BC = B // NCORES          # sequences per core
N = BC * L                # tokens per core (6400)
NWIN = 8                  # vocab windows (int16-addressable)
WINROWS = 12800
VPAD = NWIN * WINROWS     # padded table rows (102400)
CAP = 1024                # bucket slot grid per window
NVALID = 960              # indices per gather (padded; max idx/instr is 1024)
NSLOT = NWIN * CAP        # 8192 bucket slots per core
NSUB = NSLOT // 128       # 64 bucket subtiles

F32 = mybir.dt.float32
BF16 = mybir.dt.bfloat16
I16 = mybir.dt.int16
I32 = mybir.dt.int32


def build_nc():
    nc = bacc.Bacc("TRN2", target_bir_lowering=False, debug=False,
                   num_swdge_queues=4)
    ctab = nc.dram_tensor("ctab", [VPAD, D], BF16, kind="ExternalInput").ap()
    idx16 = nc.dram_tensor("idx16", [128, NSLOT // 16], I16,
                           kind="ExternalInput").ap()
    posb = nc.dram_tensor("posb", [128, NSUB, D], BF16,
                          kind="ExternalInput").ap()
    out = nc.dram_tensor("out", [128, NSUB, D], BF16,
                         kind="ExternalOutput").ap()

    SUBW = CAP // 128          # bucket subtiles per window (8)
    IDXW = CAP // 16           # idx columns per window (64)

    with (
        nc.Block() as block,
        nc.sbuf_tensor("bkt", [128, NSUB, D], BF16) as bkt,
        nc.sbuf_tensor("pos", [128, NSUB, D], BF16) as pos,
        nc.sbuf_tensor("ob", [128, NSUB, D], BF16) as ob,
        nc.sbuf_tensor("idx", [128, NSLOT // 16], I16) as idx,
        nc.semaphore("isem") as isem,       # idx load
        nc.semaphore("psem") as psem,       # pos tensor load
        nc.semaphore("vsem") as vsem,       # DVE adds done (per window)
        nc.semaphore("ssem") as ssem,       # stores done
        ExitStack() as stack,
    ):
        gsem = [stack.enter_context(nc.semaphore(f"g{c}"))
                for c in range(NWIN)]

        @block.sync
        def _(sy):
            # small gather prerequisites first, then the big pos tensor
            sy.dma_start(idx[:], idx16).then_inc(isem, 16)
            sy.dma_start(pos[:], posb).then_inc(psem, 16)
            # stores, per window as its DVE add completes
            for c in range(NWIN):
                sy.wait_ge(vsem, c + 1)
                sy.dma_start(
                    out[:, c * SUBW:(c + 1) * SUBW, :],
                    ob[:, c * SUBW:(c + 1) * SUBW, :],
                ).then_inc(ssem, 16)

        @block.gpsimd
        def _(g: bass.BassGpSimd):
            g.load_library(mlp)
            g.wait_ge(isem, 16)            # idx loaded
            for c in range(NWIN):
                g.dma_gather(
                    bkt[:, c * SUBW:(c + 1) * SUBW, :],
                    ctab[c * WINROWS:(c + 1) * WINROWS, :],
                    idx[:, c * IDXW:(c + 1) * IDXW],
                    NVALID, NVALID, D,
                    single_packet=False,
                    queue_num=c % 4,
                ).then_inc(gsem[c], 16)
            g.wait_ge(ssem, 16 * NWIN)     # keep engine alive to kernel end

        @block.vector
        def _(v):
            v.wait_ge(psem, 16)            # pos tensor loaded
            for c in range(NWIN):
                v.wait_ge(gsem[c], 16)
                v.tensor_tensor(
                    out=ob[:, c * SUBW:(c + 1) * SUBW, :],
                    in0=bkt[:, c * SUBW:(c + 1) * SUBW, :],
                    in1=pos[:, c * SUBW:(c + 1) * SUBW, :],
                    op=mybir.AluOpType.add,
                ).then_inc(vsem, 1)

    nc.compile()
    return nc


_NC_CACHE = None


def _get_nc():
    global _NC_CACHE
    if _NC_CACHE is None:
        _NC_CACHE = build_nc()
    return _NC_CACHE


def make_ctab(token_table, genre_table, token_genre_ids, genre_counts):
    """Fold the per-token genre mean into the token table (f32 math, bf16 out),
    padded to VPAD rows."""
    tok = np.asarray(token_table, dtype=np.float32)
    gt = np.asarray(genre_table, dtype=np.float32)
    gids = np.asarray(token_genre_ids)
    cnts = np.asarray(genre_counts)
    ctab = np.zeros((VPAD, D), dtype=ml_dtypes.bfloat16)
    mask8 = np.arange(MAXG)
    chunk = 25000
    for v0 in range(0, VOCAB, chunk):
        v1 = min(v0 + chunk, VOCAB)
        ge = gt[gids[v0:v1]]                                # [chunk, MAXG, D]
        m = (mask8 < cnts[v0:v1, None]).astype(np.float32)  # [chunk, MAXG]
        gm = np.einsum("vgd,vg->vd", ge, m) / cnts[v0:v1, None].astype(np.float32)
        ctab[v0:v1] = (tok[v0:v1] + gm).astype(ml_dtypes.bfloat16)
    return ctab


def prep_host_inputs(sequence, token_table, genre_table, pos_table,
                     token_genre_ids, genre_counts):
    """Host-side sharding / index prep. Returns (in_maps, tok2slot list)."""
    seq = np.ascontiguousarray(np.asarray(sequence).astype(np.int64)).reshape(B, L)
    ctab = make_ctab(token_table, genre_table, token_genre_ids, genre_counts)
    pos16 = np.asarray(pos_table, dtype=np.float32).astype(ml_dtypes.bfloat16)

    in_maps = []
    tok2slots = []
    for c in range(NCORES):
        v = seq[c * BC:(c + 1) * BC].reshape(N)         # token ids, b-major
        w = v // WINROWS                                # window of each token
        counts = np.bincount(w, minlength=NWIN)
        assert counts.max() <= NVALID, f"window overflow: {counts}"
        order = np.argsort(w, kind="stable")            # tokens by window
        # bucket slot of each token
        starts = np.arange(NWIN) * CAP
        offs = np.concatenate([np.arange(n) for n in counts]) if N else None
        slots_in_order = np.repeat(starts, counts) + offs
        tok2slot = np.empty(N, dtype=np.int64)
        tok2slot[order] = slots_in_order

        # int16 in-window indices laid out per bucket slot; pad slots (up
        # to NVALID per window) get spread-out dummy rows; slots beyond
        # NVALID in the 1024 grid are never gathered
        flat = ((np.arange(NSLOT) * 37) % WINROWS).astype(np.int16)
        flat[tok2slot] = (v - w * WINROWS).astype(np.int16)
        # wrap: idx position j -> [j%16, j//16], replicated across stripes
        tile16 = np.zeros((16, NSLOT // 16), dtype=np.int16)
        tile16[np.arange(NSLOT) % 16, np.arange(NSLOT) // 16] = flat
        idx16 = np.tile(tile16, (8, 1))

        # per-bucket-slot positional rows (0 in pad slots)
        posb_flat = np.zeros((NSLOT, D), dtype=ml_dtypes.bfloat16)
        posb_flat[tok2slot] = pos16[np.arange(N) % L]
        posb = np.ascontiguousarray(
            posb_flat.reshape(NSUB, 128, D).transpose(1, 0, 2))

        in_maps.append({
            "ctab": ctab,
            "idx16": idx16,
            "posb": posb,
        })
        tok2slots.append(tok2slot)
    return in_maps, tok2slots


def postprocess(results, tok2slots):
    """Un-permute per-core bucket-order outputs and concatenate to [B, L, D]."""
    outs = []
    for c in range(NCORES):
        o = np.asarray(results[c]["out"])               # [128, NSUB, D] bf16
        flat = o.transpose(1, 0, 2).reshape(NSLOT, D)   # slot j = i*128+p
        plain = flat[tok2slots[c]].astype(np.float32)   # [N, D]
        outs.append(plain.reshape(BC, L, D))
    return np.concatenate(outs, axis=0)


def kernel(sequence, token_table, genre_table, pos_table, token_genre_ids,
           genre_counts):
    nc = _get_nc()
    in_maps, tok2slots = prep_host_inputs(
        sequence, token_table, genre_table, pos_table, token_genre_ids,
        genre_counts)
    res = run_bass_kernel_spmd(nc, in_maps, core_ids=list(range(NCORES)))
    return postprocess(res.results, tok2slots)


# revision 24
# speedup vs baseline: 1.0477x; 1.0477x over previous
"""Trainium2 Bass kernel for nn_BERTEmbedding_65274912964883.

out[b, l, :] = token_table[seq[b, l]]
             + mean_{g in genres(seq[b, l])} genre_table[g]
             + pos_table[l]

Strategy (8 NeuronCores, SPMD, no collectives):
  - Data-parallel over batch: 256 sequences -> 32 per core (6400 tokens/core).
  - The genre mean is a function of token id only, so it is folded into the
    token table at weight-prep time: ctab[v, :] = token_table[v] +
    mean_genre[v], bf16, 256B rows.
  - The 6400-row random gather is descriptor-execution bound (~3.4ns/desc
    with all 4 SWDGE queues; ~11ns on one). The generic indirect-DMA path
    is stuck on queue 0 by ucode, so the gather uses the vectorized
    dma_gather ucode instead, spread across queues 0-3.
  - dma_gather takes int16 indices, so the vocab is split into 8 windows of
    12800 rows; the host buckets each core's token ids by window (stable
    order) and ships per-window int16 index lists, each padded to exactly
    NVALID=960 entries with spread-out dummy indices. num_idxs_reg must
    EQUAL the list's valid count or the ucode wedges; runtime counts via
    value_load work (bare only - min_val/max_val bounds wedge) but cost
    ~1us of Pool-sequencer time each, so fixed fully-padded lists win.
    Same-row dummy fetches would serialize on one HBM bank, hence the
    spread. One gather instruction per window, <= 1024 indices each
    (>1024 rows/instruction wedges the SWDGE ring).
  - The positional term is added on-device by DVE from a host-staged
    per-bucket-slot pos tensor; its load and the output stores ride the
    HWDGE queues (sync/scalar engines), overlapping the SWDGE gathers.
  - Device output is in bucket order; the host un-permutes (pure data
    movement, like the batch unshard) and converts bf16 -> f32.
"""

import numpy as np
import ml_dtypes
from contextlib import ExitStack

import concourse.bacc as bacc
import concourse.bass as bass
import concourse.mybir as mybir
from concourse.bass_utils import run_bass_kernel_spmd
from concourse.library_config import mlp

VOCAB = 100000
D = 128
MAXG = 8
B, L = 256, 200
NCORES = 8
BC = B // NCORES          # sequences per core
N = BC * L                # tokens per core (6400)
NWIN = 8                  # vocab windows (int16-addressable)
WINROWS = 12800
VPAD = NWIN * WINROWS     # padded table rows (102400)
CAP = 1024                # bucket slot grid per window
NVALID = 960              # indices per gather (padded; max idx/instr is 1024)
NSLOT = NWIN * CAP        # 8192 bucket slots per core
NSUB = NSLOT // 128       # 64 bucket subtiles

F32 = mybir.dt.float32
BF16 = mybir.dt.bfloat16
I16 = mybir.dt.int16
I32 = mybir.dt.int32


def build_nc():
    nc = bacc.Bacc("TRN2", target_bir_lowering=False, debug=False,
                   num_swdge_queues=4)
    ctab = nc.dram_tensor("ctab", [VPAD, D], BF16, kind="ExternalInput").ap()
    idx16 = nc.dram_tensor("idx16", [128, NSLOT // 16], I16,
                           kind="ExternalInput").ap()
    posb = nc.dram_tensor("posb", [128, NSUB, D], BF16,
                          kind="ExternalInput").ap()
    out = nc.dram_tensor("out", [128, NSUB, D], BF16,
                         kind="ExternalOutput").ap()

    SUBW = CAP // 128          # bucket subtiles per window (8)
    IDXW = CAP // 16           # idx columns per window (64)

    with (
        nc.Block(no_gpsimd_drain=True) as block,
        nc.sbuf_tensor("bkt", [128, NSUB, D], BF16) as bkt,
        nc.sbuf_tensor("pos", [128, NSUB, D], BF16) as pos,
        nc.sbuf_tensor("ob", [128, NSUB, D], BF16) as ob,
        nc.sbuf_tensor("idx", [128, NSLOT // 16], I16) as idx,
        nc.semaphore("isem") as isem,       # idx load
        nc.semaphore("psem") as psem,       # pos tensor load
        nc.semaphore("vsem") as vsem,       # DVE adds done (per window)
        nc.semaphore("ssem") as ssem,       # stores done
        ExitStack() as stack,
    ):
        gsem = [stack.enter_context(nc.semaphore(f"g{c}"))
                for c in range(NWIN)]

        @block.sync
        def _(sy):
            # small gather prerequisites first, then the big pos tensor
            sy.dma_start(idx[:], idx16).then_inc(isem, 16)
            sy.dma_start(pos[:], posb).then_inc(psem, 16)
            # stores, per window as its DVE add completes
            for c in range(NWIN):
                sy.wait_ge(vsem, c + 1)
                sy.dma_start(
                    out[:, c * SUBW:(c + 1) * SUBW, :],
                    ob[:, c * SUBW:(c + 1) * SUBW, :],
                ).then_inc(ssem, 16)

        @block.gpsimd
        def _(g: bass.BassGpSimd):
            g.load_library(mlp)
            g.wait_ge(isem, 16)            # idx loaded
            for c in range(NWIN):
                g.dma_gather(
                    bkt[:, c * SUBW:(c + 1) * SUBW, :],
                    ctab[c * WINROWS:(c + 1) * WINROWS, :],
                    idx[:, c * IDXW:(c + 1) * IDXW],
                    NVALID, NVALID, D,
                    queue_num=c % 4,
                ).then_inc(gsem[c], 16)
            g.wait_ge(ssem, 16 * NWIN)     # keep engine alive to kernel end

        @block.vector
        def _(v):
            v.wait_ge(psem, 16)            # pos tensor loaded
            for c in range(NWIN):
                v.wait_ge(gsem[c], 16)
                v.tensor_tensor(
                    out=ob[:, c * SUBW:(c + 1) * SUBW, :],
                    in0=bkt[:, c * SUBW:(c + 1) * SUBW, :],
                    in1=pos[:, c * SUBW:(c + 1) * SUBW, :],
                    op=mybir.AluOpType.add,
                ).then_inc(vsem, 1)

    nc.compile()
    return nc


_NC_CACHE = None


def _get_nc():
    global _NC_CACHE
    if _NC_CACHE is None:
        _NC_CACHE = build_nc()
    return _NC_CACHE


def make_ctab(token_table, genre_table, token_genre_ids, genre_counts):
    """Fold the per-token genre mean into the token table (f32 math, bf16 out),
    padded to VPAD rows."""
    tok = np.asarray(token_table, dtype=np.float32)
    gt = np.asarray(genre_table, dtype=np.float32)
    gids = np.asarray(token_genre_ids)
    cnts = np.asarray(genre_counts)
    ctab = np.zeros((VPAD, D), dtype=ml_dtypes.bfloat16)
    mask8 = np.arange(MAXG)
    chunk = 25000
    for v0 in range(0, VOCAB, chunk):
        v1 = min(v0 + chunk, VOCAB)
        ge = gt[gids[v0:v1]]                                # [chunk, MAXG, D]
        m = (mask8 < cnts[v0:v1, None]).astype(np.float32)  # [chunk, MAXG]
        gm = np.einsum("vgd,vg->vd", ge, m) / cnts[v0:v1, None].astype(np.float32)
        ctab[v0:v1] = (tok[v0:v1] + gm).astype(ml_dtypes.bfloat16)
    return ctab


def prep_host_inputs(sequence, token_table, genre_table, pos_table,
                     token_genre_ids, genre_counts):
    """Host-side sharding / index prep. Returns (in_maps, tok2slot list)."""
    seq = np.ascontiguousarray(np.asarray(sequence).astype(np.int64)).reshape(B, L)
    ctab = make_ctab(token_table, genre_table, token_genre_ids, genre_counts)
    pos16 = np.asarray(pos_table, dtype=np.float32).astype(ml_dtypes.bfloat16)

    in_maps = []
    tok2slots = []
    for c in range(NCORES):
        v = seq[c * BC:(c + 1) * BC].reshape(N)         # token ids, b-major
        w = v // WINROWS                                # window of each token
        counts = np.bincount(w, minlength=NWIN)
        assert counts.max() <= NVALID, f"window overflow: {counts}"
        order = np.argsort(w, kind="stable")            # tokens by window
        # bucket slot of each token
        starts = np.arange(NWIN) * CAP
        offs = np.concatenate([np.arange(n) for n in counts]) if N else None
        slots_in_order = np.repeat(starts, counts) + offs
        tok2slot = np.empty(N, dtype=np.int64)
        tok2slot[order] = slots_in_order

        # int16 in-window indices laid out per bucket slot; pad slots (up
        # to NVALID per window) get spread-out dummy rows; slots beyond
        # NVALID in the 1024 grid are never gathered
        flat = ((np.arange(NSLOT) * 37) % WINROWS).astype(np.int16)
        flat[tok2slot] = (v - w * WINROWS).astype(np.int16)
        # wrap: idx position j -> [j%16, j//16], replicated across stripes
        tile16 = np.zeros((16, NSLOT // 16), dtype=np.int16)
        tile16[np.arange(NSLOT) % 16, np.arange(NSLOT) // 16] = flat
        idx16 = np.tile(tile16, (8, 1))

        # per-bucket-slot positional rows (0 in pad slots)
        posb_flat = np.zeros((NSLOT, D), dtype=ml_dtypes.bfloat16)
        posb_flat[tok2slot] = pos16[np.arange(N) % L]
        posb = np.ascontiguousarray(
            posb_flat.reshape(NSUB, 128, D).transpose(1, 0, 2))

        in_maps.append({
            "ctab": ctab,
            "idx16": idx16,
            "posb": posb,
        })
        tok2slots.append(tok2slot)
    return in_maps, tok2slots


def postprocess(results, tok2slots):
    """Un-permute per-core bucket-order outputs and concatenate to [B, L, D]."""
    outs = []
    for c in range(NCORES):
        o = np.asarray(results[c]["out"])               # [128, NSUB, D] bf16
        flat = o.transpose(1, 0, 2).reshape(NSLOT, D)   # slot j = i*128+p
        plain = flat[tok2slots[c]].astype(np.float32)   # [N, D]
        outs.append(plain.reshape(BC, L, D))
    return np.concatenate(outs, axis=0)


def kernel(sequence, token_table, genre_table, pos_table, token_genre_ids,
           genre_counts):
    nc = _get_nc()
    in_maps, tok2slots = prep_host_inputs(
        sequence, token_table, genre_table, pos_table, token_genre_ids,
        genre_counts)
    res = run_bass_kernel_spmd(nc, in_maps, core_ids=list(range(NCORES)))
    return postprocess(res.results, tok2slots)
